# revision 9
# baseline (speedup 1.0000x reference)
"""Trainium2 Bass kernel for nn_ComplexNet: out = x @ M_r.T

Reference math: x_imag = 0, so only M_r (the real coefficient matrix,
[2, 10], built from psi/A via a tiny einsum) matters:
    out[t, k] = sum_a x[t, a] * M_r[k, a]

The problem is pure memory streaming; the only lever that matters is HBM
bytes moved.  The rel-err budget (2e-2) is ~30x looser than fp16
round-off, so the host stages x in fp16 and reads the result back as
int8 (scale folded into the weights): 11.2 MB/core instead of
24.6 MB/core, against a ~360 GB/s per-core DMA bus.

Measured-on-HW design notes:
  - DMA descriptors need >=4 KB to reach bus rate (1 KB descriptors ran
    231 GB/s, 10 KB runs 352-362 GB/s), so the host packs x per-core in
    chunks [128, 5, F_ch] (j-major) and each chunk is ONE DMA whose
    per-partition run is contiguous: 2-tile chunks = 10 KB descriptors.
  - Partition p = 2g+c (g = row-group 0..63, c = pair lane); column
    (j, f) holds x[row = f*64 + g, feature = 2j+c] in fp16.
  - Stationary weights W_j [128, 128] (2x2 diagonal blocks):
    W_j[2g+c, 2g+k] = M[k, 2j+c] * s_k, with s_k = 127/(6.5*||M_k||)
    so PSUM values span +-127.  Per 512-col PSUM tile, 5 fp16 matmuls
    accumulate psum[2g+k, f]; weights load first on the Sync queue
    (tiny; on the gpsimd SWDGE it landed ~16 us late and stalled PE).
  - ACT evicts each PSUM bank directly to int8 (round-to-nearest-even +
    saturate, verified on HW) and its HWDGE queue stores ~2-chunk
    batches; host divides by s_k and un-permutes.
  - Chunk sizes [133|512|1024*6|512|512]: small head so compute starts
    early, small tail because the last chunk's matmul+evict+store can't
    overlap anything.

kernel(**inputs) takes the FULL unsharded inputs, returns the FULL
[4_000_000, 2] float32 output.
"""

import sys

import numpy as np

if "/opt/trn_rl_repo" not in sys.path:
    sys.path.insert(0, "/opt/trn_rl_repo")

from contextlib import ExitStack

import concourse.bacc as bacc
import concourse.tile as tile
from concourse import mybir
from concourse.bass_utils import run_bass_kernel_spmd

T = 4_000_000
N_FEAT = 10
N_CORES = 8
P = 128
G = 64            # row groups (rows per moving column)
NJ = 5            # feature pairs

F_TOTAL = 7813    # moving columns per core
R = G * F_TOTAL   # 500_032 rows per core
T_PAD = R * N_CORES  # 4_000_256

# PSUM-bank-sized tiles (<=512 f32 columns), grouped into chunks that
# share one input DMA; store groups share one output DMA.
CHUNKS = ([[133], [512]] + [[512, 512]] * 6 + [[512], [512]])
F_CH = [sum(c) for c in CHUNKS]
assert sum(F_CH) == F_TOTAL
# chunk indices after which the accumulated evictions are stored
STORE_GROUPS = [[0, 1], [2, 3], [4, 5], [6, 7], [8, 9]]

DT = mybir.dt.float16
DT32 = mybir.dt.float32
DT8 = mybir.dt.int8
OUT_SIGMA = 6.5   # psum scaled to +-127 at OUT_SIGMA * ||M_k||

_CACHE = {}


def _build():
    if "nc" in _CACHE:
        return _CACHE["nc"]
    nc = bacc.Bacc("TRN2", target_bir_lowering=False, debug=False,
                   num_devices=N_CORES)
    x_d = nc.dram_tensor("x", [P, NJ * F_TOTAL], DT, kind="ExternalInput")
    w_d = nc.dram_tensor("w", [P, NJ * P], DT, kind="ExternalInput")
    o_d = nc.dram_tensor("out", [P, F_TOTAL], DT8, kind="ExternalOutput")

    x2 = x_d.ap()   # [p, chunk-packed cols]
    o2 = o_d.ap()   # [p, f] (f-major, global col order)

    with tile.TileContext(nc) as tc, ExitStack() as ctx:
        consts = ctx.enter_context(tc.tile_pool(name="consts", bufs=1))
        xpool = ctx.enter_context(tc.tile_pool(name="xp", bufs=2))
        opool = ctx.enter_context(tc.tile_pool(name="op", bufs=2))
        psum = ctx.enter_context(tc.tile_pool(name="ps", bufs=6, space="PSUM"))

        # weights first on the Sync queue: tiny (0.5 us) and the PE can't
        # start without them
        w_sb = consts.tile([P, NJ * P], DT)
        nc.sync.dma_start(w_sb[:], w_d.ap())

        grp_of_chunk = {}
        for gi_, chs in enumerate(STORE_GROUPS):
            for c in chs:
                grp_of_chunk[c] = gi_

        x_off = 0
        g_off = 0
        psum_i = 0
        o_sb = None
        o_lo = 0    # global col offset where current staging tile starts
        o_fill = 0  # cols filled in current staging tile
        for ci, tiles in enumerate(CHUNKS):
            FC = F_CH[ci]
            x_sb = xpool.tile([P, NJ * FC], DT, name=f"x_{ci}")
            nc.sync.dma_start(x_sb[:], x2[:, x_off:x_off + NJ * FC])

            gi_ = grp_of_chunk[ci]
            if o_sb is None:
                gcols = sum(F_CH[c] for c in STORE_GROUPS[gi_])
                o_sb = opool.tile([P, gcols], DT8, name=f"o_{gi_}")
                o_lo = g_off
                o_fill = 0

            b_off = 0
            for FT in tiles:
                ps = psum.tile([P, FT], DT32, name=f"ps_{psum_i}", tag="ps")
                psum_i += 1
                for j in range(NJ):
                    nc.tensor.matmul(
                        ps[:],
                        w_sb[:, j * P:(j + 1) * P],
                        x_sb[:, j * FC + b_off:j * FC + b_off + FT],
                        start=(j == 0), stop=(j == NJ - 1),
                    )
                # ACT: psum f32 -> int8 (round-to-nearest-even, saturating)
                nc.scalar.copy(o_sb[:, o_fill:o_fill + FT], ps[:])
                o_fill += FT
                b_off += FT

            if ci == STORE_GROUPS[gi_][-1]:
                # ACT HWDGE queue: keeps Sync free for input prefetch
                nc.scalar.dma_start(o2[:, o_lo:o_lo + o_fill], o_sb[:])
                o_sb = None
            g_off += FC
            x_off += NJ * FC

    nc.compile()
    _CACHE["nc"] = nc
    return nc


def _host_m(psi_real, psi_imag, A_real, A_imag):
    """M_r in float64: the coefficient matrix multiplying x_real."""
    pr = psi_real.astype(np.float64)
    pi = psi_imag.astype(np.float64)
    Ar = A_real.astype(np.float64)
    Ai = A_imag.astype(np.float64)

    def mat(p1, A, p2):
        return np.einsum("i,kija,j->ka", p1, A, p2)

    M = (mat(pr, Ar, pr) - mat(pi, Ai, pr)
         - mat(pr, Ar, pi) + mat(pi, Ai, pi))
    return M  # [2, 10] f64


def kernel(x, psi_real, psi_imag, A_real, A_imag, _trace=False):
    M = _host_m(psi_real, psi_imag, A_real, A_imag)
    s = 127.0 / (OUT_SIGMA * np.linalg.norm(M, axis=1))  # [2]

    # logical layout: Y[core, p=2g+c, j, f] = x[core*R + f*64 + g, 2j+c]
    xq = np.zeros((T_PAD, N_FEAT), dtype=np.float16)
    xq[:T] = x
    Y = (xq.reshape(N_CORES, F_TOTAL, G, NJ, 2)
         .transpose(0, 2, 4, 3, 1)
         .reshape(N_CORES, P, NJ, F_TOTAL))
    # pack into per-chunk [p, j-major] blocks
    parts = []
    off = 0
    for FC in F_CH:
        parts.append(Y[:, :, :, off:off + FC].reshape(N_CORES, P, NJ * FC))
        off += FC
    X = np.ascontiguousarray(np.concatenate(parts, axis=2))

    # W_j[2g+c, j*128 + 2g+k] = M[k, 2j+c] * s_k
    W = np.zeros((P, NJ * P), dtype=np.float16)
    g = np.arange(G)
    for j in range(NJ):
        for c in range(2):
            for k in range(2):
                W[2 * g + c, j * P + 2 * g + k] = np.float16(M[k, 2 * j + c] * s[k])

    nc = _build()
    in_maps = [{"x": X[c], "w": W} for c in range(N_CORES)]
    res = run_bass_kernel_spmd(nc, in_maps, core_ids=list(range(N_CORES)),
                               trace=_trace)
    # out_dev[core, 2g+k, f] = round(out[core*R + f*64 + g, k] * s_k)
    O = np.stack([res.results[c]["out"] for c in range(N_CORES)])
    O = O.reshape(N_CORES, G, 2, F_TOTAL).astype(np.float32)
    O /= s[None, None, :, None]
    out = (O.transpose(0, 3, 1, 2).reshape(T_PAD, 2)[:T])
    if _trace:
        kernel.last_results = res
    return out


# revision 10
# speedup vs baseline: 1.0021x; 1.0021x over previous
"""Trainium2 Bass kernel for nn_ComplexNet: out = x @ M_r.T

Reference math: x_imag = 0, so only M_r (the real coefficient matrix,
[2, 10], built from psi/A via a tiny einsum) matters:
    out[t, k] = sum_a x[t, a] * M_r[k, a]

The problem is pure memory streaming; the only lever that matters is HBM
bytes moved.  The rel-err budget (2e-2) is ~30x looser than fp16
round-off, so the host stages x in fp16 and reads the result back as
int8 (scale folded into the weights): 11.2 MB/core instead of
24.6 MB/core, against a ~360 GB/s per-core DMA bus.

Measured-on-HW design notes:
  - DMA descriptors need >=4 KB to reach bus rate (1 KB descriptors ran
    231 GB/s, 10 KB runs 352 GB/s), so the host packs x per-core in
    chunks [128, 5, F_ch] (j-major) and each chunk is ONE DMA whose
    per-partition run is contiguous.  The stationary weights ride in
    front of chunk 0 (one DMA, one semaphore, PE starts sooner).
  - Partition p = 2g+c (g = row-group 0..63, c = pair lane); column
    (j, f) holds x[row = f*64 + g, feature = 2j+c] in fp16.
  - Stationary weights W_j [128, 128] (2x2 diagonal blocks):
    W_j[2g+c, 2g+k] = M[k, 2j+c] * s_k, with s_k = 127/(6.5*||M_k||)
    so PSUM values span +-127.
  - Per 512-col PSUM tile, 5 fp16 matmuls accumulate psum[2g+k, f];
    ACT evicts each bank directly to int8 (round-to-nearest-even +
    saturate, verified on HW) and its HWDGE queue stores multi-chunk
    batches; host divides by s_k and un-permutes.
  - Chunk sizes [133|512|1024*6|512|512]: small head so compute starts
    early, small tail because the last chunk's matmul+evict+store can't
    overlap anything.
  - Single SBUF tile pool with per-role tags (x rotates 3-deep) - extra
    pools each add an all-engine exit barrier to the measured window.

kernel(**inputs) takes the FULL unsharded inputs, returns the FULL
[4_000_000, 2] float32 output.
"""

import sys

import numpy as np

if "/opt/trn_rl_repo" not in sys.path:
    sys.path.insert(0, "/opt/trn_rl_repo")

from contextlib import ExitStack

import concourse.bacc as bacc
import concourse.tile as tile
from concourse import mybir
from concourse.bass_utils import run_bass_kernel_spmd

T = 4_000_000
N_FEAT = 10
N_CORES = 8
P = 128
G = 64            # row groups (rows per moving column)
NJ = 5            # feature pairs

F_TOTAL = 7813    # moving columns per core
R = G * F_TOTAL   # 500_032 rows per core
T_PAD = R * N_CORES  # 4_000_256

# PSUM-bank-sized tiles (<=512 f32 columns), grouped into chunks that
# share one input DMA; store groups share one output DMA.
CHUNKS = ([[133], [512]] + [[512, 512]] * 6 + [[512], [512]])
F_CH = [sum(c) for c in CHUNKS]
assert sum(F_CH) == F_TOTAL
# chunk indices after which the accumulated evictions are stored
STORE_GROUPS = [[0, 1], [2, 3], [4, 5], [6, 7], [8], [9]]

DT = mybir.dt.float16
DT32 = mybir.dt.float32
DT8 = mybir.dt.int8
OUT_SIGMA = 6.5   # psum scaled to +-127 at OUT_SIGMA * ||M_k||
WCOLS = NJ * P    # weight columns riding in front of chunk 0

_CACHE = {}


def _build():
    if "nc" in _CACHE:
        return _CACHE["nc"]
    nc = bacc.Bacc("TRN2", target_bir_lowering=False, debug=False,
                   num_devices=N_CORES)
    x_d = nc.dram_tensor("x", [P, WCOLS + NJ * F_TOTAL], DT,
                         kind="ExternalInput")
    o_d = nc.dram_tensor("out", [P, F_TOTAL], DT8, kind="ExternalOutput")

    x2 = x_d.ap()   # [p, w + chunk-packed cols]
    o2 = o_d.ap()   # [p, f] (f-major, global col order)

    with tile.TileContext(nc) as tc, ExitStack() as ctx:
        sbuf = ctx.enter_context(tc.tile_pool(name="sb", bufs=3))
        psum = ctx.enter_context(tc.tile_pool(name="ps", bufs=6, space="PSUM"))

        grp_of_chunk = {}
        for gi_, chs in enumerate(STORE_GROUPS):
            for c in chs:
                grp_of_chunk[c] = gi_

        x_off = 0
        g_off = 0
        psum_i = 0
        w_sb = None
        o_sb = None
        o_lo = 0    # global col offset where current staging tile starts
        o_fill = 0  # cols filled in current staging tile
        for ci, tiles in enumerate(CHUNKS):
            FC = F_CH[ci]
            if ci == 0:
                # head chunk carries the weights: one DMA, one semaphore
                head = sbuf.tile([P, WCOLS + NJ * FC], DT, name="head",
                                 tag="head")
                nc.sync.dma_start(head[:], x2[:, :WCOLS + NJ * FC])
                w_sb = head[:, :WCOLS]
                x_sb = head[:, WCOLS:]
                x_off = WCOLS + NJ * FC
            else:
                xt = sbuf.tile([P, NJ * FC], DT, name=f"x_{ci}", tag="x")
                nc.sync.dma_start(xt[:], x2[:, x_off:x_off + NJ * FC])
                x_sb = xt[:]
                x_off += NJ * FC

            gi_ = grp_of_chunk[ci]
            if o_sb is None:
                gcols = sum(F_CH[c] for c in STORE_GROUPS[gi_])
                o_sb = sbuf.tile([P, gcols], DT8, name=f"o_{gi_}", tag="o")
                o_lo = g_off
                o_fill = 0

            b_off = 0
            for FT in tiles:
                ps = psum.tile([P, FT], DT32, name=f"ps_{psum_i}", tag="ps")
                psum_i += 1
                for j in range(NJ):
                    nc.tensor.matmul(
                        ps[:],
                        w_sb[:, j * P:(j + 1) * P],
                        x_sb[:, j * FC + b_off:j * FC + b_off + FT],
                        start=(j == 0), stop=(j == NJ - 1),
                    )
                # ACT: psum f32 -> int8 (round-to-nearest-even, saturating)
                nc.scalar.copy(o_sb[:, o_fill:o_fill + FT], ps[:])
                o_fill += FT
                b_off += FT

            if ci == STORE_GROUPS[gi_][-1]:
                # ACT HWDGE queue: keeps Sync free for input prefetch
                nc.scalar.dma_start(o2[:, o_lo:o_lo + o_fill], o_sb[:])
                o_sb = None
            g_off += FC

    nc.compile()
    _CACHE["nc"] = nc
    return nc


def _host_m(psi_real, psi_imag, A_real, A_imag):
    """M_r in float64: the coefficient matrix multiplying x_real."""
    pr = psi_real.astype(np.float64)
    pi = psi_imag.astype(np.float64)
    Ar = A_real.astype(np.float64)
    Ai = A_imag.astype(np.float64)

    def mat(p1, A, p2):
        return np.einsum("i,kija,j->ka", p1, A, p2)

    M = (mat(pr, Ar, pr) - mat(pi, Ai, pr)
         - mat(pr, Ar, pi) + mat(pi, Ai, pi))
    return M  # [2, 10] f64


def kernel(x, psi_real, psi_imag, A_real, A_imag, _trace=False):
    M = _host_m(psi_real, psi_imag, A_real, A_imag)
    s = 127.0 / (OUT_SIGMA * np.linalg.norm(M, axis=1))  # [2]

    # logical layout: Y[core, p=2g+c, j, f] = x[core*R + f*64 + g, 2j+c]
    xq = np.zeros((T_PAD, N_FEAT), dtype=np.float16)
    xq[:T] = x
    Y = (xq.reshape(N_CORES, F_TOTAL, G, NJ, 2)
         .transpose(0, 2, 4, 3, 1)
         .reshape(N_CORES, P, NJ, F_TOTAL))

    # W_j[2g+c, j*128 + 2g+k] = M[k, 2j+c] * s_k
    W = np.zeros((P, NJ * P), dtype=np.float16)
    g = np.arange(G)
    for j in range(NJ):
        for c in range(2):
            for k in range(2):
                W[2 * g + c, j * P + 2 * g + k] = np.float16(M[k, 2 * j + c] * s[k])

    # pack: weights, then per-chunk [p, j-major] blocks
    parts = [np.broadcast_to(W, (N_CORES, P, NJ * P))]
    off = 0
    for FC in F_CH:
        parts.append(Y[:, :, :, off:off + FC].reshape(N_CORES, P, NJ * FC))
        off += FC
    X = np.ascontiguousarray(np.concatenate(parts, axis=2))

    nc = _build()
    in_maps = [{"x": X[c]} for c in range(N_CORES)]
    res = run_bass_kernel_spmd(nc, in_maps, core_ids=list(range(N_CORES)),
                               trace=_trace)
    # out_dev[core, 2g+k, f] = round(out[core*R + f*64 + g, k] * s_k)
    O = np.stack([res.results[c]["out"] for c in range(N_CORES)])
    O = O.reshape(N_CORES, G, 2, F_TOTAL).astype(np.float32)
    O /= s[None, None, :, None]
    out = (O.transpose(0, 3, 1, 2).reshape(T_PAD, 2)[:T])
    if _trace:
        kernel.last_results = res
    return out


# revision 11
# speedup vs baseline: 1.0307x; 1.0286x over previous
"""Trainium2 Bass kernel for nn_ComplexNet: out = x @ M_r.T

Reference math: x_imag = 0, so only M_r (the real coefficient matrix,
[2, 10], built from psi/A via a tiny einsum) matters:
    out[t, k] = sum_a x[t, a] * M_r[k, a]

The problem is pure memory streaming; the only lever that matters is HBM
bytes moved.  The rel-err budget (2e-2) is ~30x looser than fp16
round-off, so the host stages x in fp16 and reads the result back as
int8 (scale folded into the weights): 11.2 MB/core instead of
24.6 MB/core, against a ~360 GB/s per-core DMA bus.

Measured-on-HW design notes:
  - DMA descriptors need >=4 KB to reach bus rate (1 KB descriptors ran
    231 GB/s, 10 KB runs 352 GB/s), so the host packs x per-core in
    chunks [128, 5, F_ch] (j-major) and each chunk is ONE DMA whose
    per-partition run is contiguous.  The stationary weights ride in
    front of chunk 0 (one DMA, one semaphore, PE starts sooner).
  - Partition p = 2g+c (g = row-group 0..63, c = pair lane); column
    (j, f) holds x[row = f*64 + g, feature = 2j+c] in fp16.
  - Stationary weights W_j [128, 128] (2x2 diagonal blocks):
    W_j[2g+c, 2g+k] = M[k, 2j+c] * s_k, with s_k = 127/(6.5*||M_k||)
    so PSUM values span +-127.
  - Per 512-col PSUM tile, 5 fp16 matmuls accumulate psum[2g+k, f];
    ACT evicts each bank directly to int8 (round-to-nearest-even +
    saturate, verified on HW) and its HWDGE queue stores multi-chunk
    batches; host divides by s_k and un-permutes.
  - Chunk sizes [133|512|1024*6|512|512]: small head so compute starts
    early, small tail because the last chunk's matmul+evict+store can't
    overlap anything.
  - Single SBUF tile pool with per-role tags (x rotates 3-deep) - extra
    pools each add an all-engine exit barrier to the measured window.

kernel(**inputs) takes the FULL unsharded inputs, returns the FULL
[4_000_000, 2] float32 output.
"""

import sys

import numpy as np

if "/opt/trn_rl_repo" not in sys.path:
    sys.path.insert(0, "/opt/trn_rl_repo")

from contextlib import ExitStack

import concourse.bacc as bacc
import concourse.tile as tile
from concourse import mybir
from concourse.bass_utils import run_bass_kernel_spmd

T = 4_000_000
N_FEAT = 10
N_CORES = 8
P = 128
G = 64            # row groups (rows per moving column)
NJ = 5            # feature pairs

F_TOTAL = 7813    # moving columns per core
R = G * F_TOTAL   # 500_032 rows per core
T_PAD = R * N_CORES  # 4_000_256

# PSUM-bank-sized tiles (<=512 f32 columns), grouped into chunks that
# share one input DMA; store groups share one output DMA.
CHUNKS = ([[133], [512]] + [[512, 512]] * 6 + [[512], [512]])
F_CH = [sum(c) for c in CHUNKS]
assert sum(F_CH) == F_TOTAL
# chunk indices after which the accumulated evictions are stored
STORE_GROUPS = [[0, 1], [2, 3], [4, 5], [6, 7], [8, 9]]

DT = mybir.dt.float16
DT32 = mybir.dt.float32
DT8 = mybir.dt.int8
OUT_SIGMA = 6.5   # psum scaled to +-127 at OUT_SIGMA * ||M_k||
WCOLS = NJ * P    # weight columns riding in front of chunk 0

_CACHE = {}


def _build():
    if "nc" in _CACHE:
        return _CACHE["nc"]
    nc = bacc.Bacc("TRN2", target_bir_lowering=False, debug=False,
                   num_devices=N_CORES)
    x_d = nc.dram_tensor("x", [P, WCOLS + NJ * F_TOTAL], DT,
                         kind="ExternalInput")
    o_d = nc.dram_tensor("out", [P, F_TOTAL], DT, kind="ExternalOutput")

    x2 = x_d.ap()   # [p, w + chunk-packed cols]
    o2 = o_d.ap()   # [p, f] (f-major, global col order)

    with tile.TileContext(nc) as tc, ExitStack() as ctx:
        sbuf = ctx.enter_context(tc.tile_pool(name="sb", bufs=3))
        psum = ctx.enter_context(tc.tile_pool(name="ps", bufs=6, space="PSUM"))

        grp_of_chunk = {}
        for gi_, chs in enumerate(STORE_GROUPS):
            for c in chs:
                grp_of_chunk[c] = gi_

        x_off = 0
        g_off = 0
        psum_i = 0
        w_sb = None
        o_sb = None
        o_lo = 0    # global col offset where current staging tile starts
        o_fill = 0  # cols filled in current staging tile
        for ci, tiles in enumerate(CHUNKS):
            FC = F_CH[ci]
            if ci == 0:
                # head chunk carries the weights: one DMA, one semaphore
                head = sbuf.tile([P, WCOLS + NJ * FC], DT, name="head",
                                 tag="head")
                nc.sync.dma_start(head[:], x2[:, :WCOLS + NJ * FC])
                w_sb = head[:, :WCOLS]
                x_sb = head[:, WCOLS:]
                x_off = WCOLS + NJ * FC
            else:
                xt = sbuf.tile([P, NJ * FC], DT, name=f"x_{ci}", tag="x")
                nc.sync.dma_start(xt[:], x2[:, x_off:x_off + NJ * FC])
                x_sb = xt[:]
                x_off += NJ * FC

            gi_ = grp_of_chunk[ci]
            if o_sb is None:
                gcols = sum(F_CH[c] for c in STORE_GROUPS[gi_])
                o_sb = sbuf.tile([P, gcols], DT, name=f"o_{gi_}", tag="o")
                o_lo = g_off
                o_fill = 0

            b_off = 0
            for FT in tiles:
                ps = psum.tile([P, FT], DT32, name=f"ps_{psum_i}", tag="ps")
                psum_i += 1
                for j in range(NJ):
                    nc.tensor.matmul(
                        ps[:],
                        w_sb[:, j * P:(j + 1) * P],
                        x_sb[:, j * FC + b_off:j * FC + b_off + FT],
                        start=(j == 0), stop=(j == NJ - 1),
                    )
                # ACT: psum f32 -> fp16
                nc.scalar.copy(o_sb[:, o_fill:o_fill + FT], ps[:])
                o_fill += FT
                b_off += FT

            if ci == STORE_GROUPS[gi_][-1]:
                # ACT HWDGE queue: keeps Sync free for input prefetch
                nc.scalar.dma_start(o2[:, o_lo:o_lo + o_fill], o_sb[:])
                o_sb = None
            g_off += FC

    nc.compile()
    _CACHE["nc"] = nc
    return nc


def _host_m(psi_real, psi_imag, A_real, A_imag):
    """M_r in float64: the coefficient matrix multiplying x_real."""
    pr = psi_real.astype(np.float64)
    pi = psi_imag.astype(np.float64)
    Ar = A_real.astype(np.float64)
    Ai = A_imag.astype(np.float64)

    def mat(p1, A, p2):
        return np.einsum("i,kija,j->ka", p1, A, p2)

    M = (mat(pr, Ar, pr) - mat(pi, Ai, pr)
         - mat(pr, Ar, pi) + mat(pi, Ai, pi))
    return M  # [2, 10] f64


def kernel(x, psi_real, psi_imag, A_real, A_imag, _trace=False):
    M = _host_m(psi_real, psi_imag, A_real, A_imag)

    # logical layout: Y[core, p=2g+c, j, f] = x[core*R + f*64 + g, 2j+c]
    xq = np.zeros((T_PAD, N_FEAT), dtype=np.float16)
    xq[:T] = x
    Y = (xq.reshape(N_CORES, F_TOTAL, G, NJ, 2)
         .transpose(0, 2, 4, 3, 1)
         .reshape(N_CORES, P, NJ, F_TOTAL))

    # W_j[2g+c, j*128 + 2g+k] = M[k, 2j+c] * s_k
    W = np.zeros((P, NJ * P), dtype=np.float16)
    g = np.arange(G)
    for j in range(NJ):
        for c in range(2):
            for k in range(2):
                W[2 * g + c, j * P + 2 * g + k] = np.float16(M[k, 2 * j + c])

    # pack: weights, then per-chunk [p, j-major] blocks
    parts = [np.broadcast_to(W, (N_CORES, P, NJ * P))]
    off = 0
    for FC in F_CH:
        parts.append(Y[:, :, :, off:off + FC].reshape(N_CORES, P, NJ * FC))
        off += FC
    X = np.ascontiguousarray(np.concatenate(parts, axis=2))

    nc = _build()
    in_maps = [{"x": X[c]} for c in range(N_CORES)]
    res = run_bass_kernel_spmd(nc, in_maps, core_ids=list(range(N_CORES)),
                               trace=_trace)
    # out_dev[core, 2g+k, f] = out[core*R + f*64 + g, k]
    O = np.stack([res.results[c]["out"] for c in range(N_CORES)])
    O = O.reshape(N_CORES, G, 2, F_TOTAL).astype(np.float32)
    out = (O.transpose(0, 3, 1, 2).reshape(T_PAD, 2)[:T])
    if _trace:
        kernel.last_results = res
    return out
